# revision 13
# baseline (speedup 1.0000x reference)
"""DonutSwin window self-attention on 8 Trainium2 NeuronCores.

Strategy (data-parallel over windows, 512 windows/core):
- Host: shard hidden_states over cores, pre-transpose + cast each shard to
  xT f16 [512, 25088] (feature-major), fold the 1/sqrt(hd) scale into Wq,
  cast weights to f16, precompute exp(rel-pos-bias)^T tiles (head-paired
  layout: pair (hp, hp+8) shares a PE row strip).
- Device per core, per 8-window block:
  * qT/kT = W^T @ xT via f16 matmuls (feature-major, head-dim on
    partitions); FWL hides the 128-col weight loads. oc order 0,2,1,3 so
    early head-quads unblock scores sooner.
  * v = x @ Wv via f16 matmuls over 49-token windows into a
    {win0: partitions 0-48, win1: 64-112} pair layout (2x col-tile
    concurrency via tile_position).
  * scores^T[k,q]: per head-pair (hp, hp+8) ONE PSUM bank [128, 2, 4, 49]
    holds all 8 windows x 2 heads; both heads use row strip 32*(hp%4), so
    the one-row-strip-per-bank PE rule holds; win0/win1 go to col halves.
  * softmax: one exp per head-pair on ACT (FD=392); multiply by
    exp(bias)^T on GPSIMD (frees DVE); row-sums via an appended
    ones-column in V.
  * ctx = P@V per (window-pair, head-half) into 2-bank PSUM tiles
    (win0 -> bank A, win1 -> bank B), normalized on DVE by recip sums.
- PSUM pools double as scheduling fences (mirrors the tuned baseline):
  one 4-buf pool shared by qk+scores, one 2-buf 2-bank pool by v+ctx.
- Output gathered to [4096, 49, 512] fp32.
"""

import numpy as np

WIN = 7
DIM = 512
HEADS = 16
HD = DIM // HEADS  # 32
B = 4096
N = WIN * WIN  # 49
NCORES = 8
BC = B // NCORES  # 512 windows per core
T = BC * N  # 25088 tokens per core
OCT = 8  # windows per block
NOCT = BC // OCT  # 64
TOK_OCT = OCT * N  # 392
PAIRS = OCT // 2  # 4 window-pairs per block

_NC_CACHE = {}
CFG = {
    "ebmul": "vector",  # engine for exp(bias) multiply: gpsimd | vector
    "qcopy": "scalar",
    "kcopy": "vector",
    "vcopy": "scalar",
    "xt_bufs": 4,
    "qk_bufs": 3,
    "e_bufs": 12,
    "o_bufs": 6,
    "v_bufs": 8,
    "ps1": 4,
    "ps2": 2,
}


def _build_nc(has_vbias, noct=NOCT):
    import concourse.bass as bass  # noqa: F401
    import concourse.tile as tile
    from concourse import bacc, mybir

    f32 = mybir.dt.float32
    f16 = mybir.dt.float16
    AFT = mybir.ActivationFunctionType

    nc = bacc.Bacc("TRN2", target_bir_lowering=False, debug=False)

    T_ = noct * TOK_OCT
    xT_d = nc.dram_tensor("xt", [DIM, T_], f16, kind="ExternalInput")
    wq_d = nc.dram_tensor("wq", [DIM, DIM], f16, kind="ExternalInput")
    wk_d = nc.dram_tensor("wk", [DIM, DIM], f16, kind="ExternalInput")
    wv_d = nc.dram_tensor("wv", [DIM, DIM], f16, kind="ExternalInput")
    bq_d = nc.dram_tensor("bq", [128, 4], f32, kind="ExternalInput")
    bk_d = nc.dram_tensor("bk", [128, 4], f32, kind="ExternalInput")
    bv_d = nc.dram_tensor("bv", [1, DIM], f16, kind="ExternalInput")
    # exp(bias)^T head-paired: [p, hp, j, q] = exp(bias[h=hp+8j].T)[k=p, q]
    # rows 0-48 and 64-112 both hold the same table (win0/win1 halves).
    eb_d = nc.dram_tensor("expb", [128, HEADS // 2, 2, N], f16, kind="ExternalInput")
    out_d = nc.dram_tensor("out", [T_, DIM], f16, kind="ExternalOutput")

    with tile.TileContext(nc) as tc:
        with (
            tc.tile_pool(name="consts", bufs=1) as consts,
            tc.tile_pool(name="xt", bufs=CFG["xt_bufs"]) as xt_pool,
            tc.tile_pool(name="qk", bufs=CFG["qk_bufs"]) as qk_pool,
            tc.tile_pool(name="vsb", bufs=CFG["v_bufs"]) as v_pool,
            tc.tile_pool(name="esb", bufs=CFG["e_bufs"]) as e_pool,
            tc.tile_pool(name="osb", bufs=CFG["o_bufs"]) as o_pool,
            tc.tile_pool(name="small", bufs=10) as small,
            tc.tile_pool(name="ps1", bufs=CFG["ps1"], space="PSUM") as ps1,
            tc.tile_pool(name="ps2", bufs=CFG["ps2"], space="PSUM") as ps2,
        ):
            wq_sb = consts.tile([128, 4, DIM], f16, tag="wq")
            wk_sb = consts.tile([128, 4, DIM], f16, tag="wk")
            wv_sb = consts.tile([128, 4, DIM], f16, tag="wv")
            for w_sb, w_d in ((wq_sb, wq_d), (wk_sb, wk_d), (wv_sb, wv_d)):
                nc.sync.dma_start(
                    out=w_sb, in_=w_d[:, :].rearrange("(i p) o -> p i o", p=128)
                )
            bq_sb = consts.tile([128, 4], f32, tag="bq")
            bk_sb = consts.tile([128, 4], f32, tag="bk")
            nc.sync.dma_start(out=bq_sb, in_=bq_d[:, :])
            nc.sync.dma_start(out=bk_sb, in_=bk_d[:, :])
            eb_sb = consts.tile([128, HEADS // 2, 2, N], f16, tag="eb")
            nc.sync.dma_start(out=eb_sb, in_=eb_d[:, :, :, :])
            if has_vbias:
                onep_sb = consts.tile([1, 2 * N], f16, tag="onep")
                nc.vector.memset(onep_sb, 1.0)
                bv_sb = consts.tile([1, DIM], f16, tag="bv")
                nc.sync.dma_start(out=bv_sb, in_=bv_d[:, :])

            OC_ORDER = (0, 2, 1, 3)  # early quads (0, 2) unblock hp 0-3 first

            for oct_i in range(noct):
                t0 = oct_i * TOK_OCT
                xt = xt_pool.tile([128, 4, TOK_OCT], f16, tag="xt")
                nc.sync.dma_start(
                    out=xt,
                    in_=xT_d[:, t0 : t0 + TOK_OCT].rearrange("(i p) t -> p i t", p=128),
                )

                # --- q^T / k^T projections (feature-major, f16 + FWL) ---
                qT = qk_pool.tile([128, 4, TOK_OCT], f16, tag="qT")
                kT = qk_pool.tile([128, 4, TOK_OCT], f16, tag="kT")
                for oc in OC_ORDER:
                    for dst, w_sb, b_sb, on_act in (
                        (qT, wq_sb, bq_sb, True),
                        (kT, wk_sb, bk_sb, False),
                    ):
                        ps = ps1.tile([128, TOK_OCT], f32, tag="ps1")
                        for ic in range(4):
                            nc.tensor.matmul(
                                ps[:, :],
                                w_sb[:, ic, oc * 128 : (oc + 1) * 128],
                                xt[:, ic, :],
                                start=(ic == 0),
                                stop=(ic == 3),
                            )
                        if on_act:
                            if has_vbias:
                                nc.scalar.activation(
                                    out=dst[:, oc, :],
                                    in_=ps[:, :],
                                    func=AFT.Identity,
                                    bias=b_sb[:, oc : oc + 1],
                                )
                            else:
                                nc.scalar.activation(
                                    out=dst[:, oc, :], in_=ps[:, :], func=AFT.Copy
                                )
                        elif has_vbias:
                            nc.vector.tensor_scalar_add(
                                dst[:, oc, :], ps[:, :], b_sb[:, oc : oc + 1]
                            )
                        else:
                            nc.vector.tensor_copy(dst[:, oc, :], ps[:, :])

                # --- v projection: col-tiled into {0-48, 64-112} pair layout ---
                v_tiles = []
                for pr in range(PAIRS):
                    v2 = ps2.tile([128, 2, DIM], f32, tag="ps2")
                    v_ps = v2[:, 0, :]
                    for ic in range(4):
                        for w01 in (0, 1):
                            c0 = (pr * 2 + w01) * N
                            nc.tensor.matmul(
                                v_ps[w01 * 64 : w01 * 64 + N, :],
                                xt[:, ic, c0 : c0 + N],
                                wv_sb[:, ic, :],
                                start=(ic == 0),
                                stop=(ic == 3) and not has_vbias,
                                tile_position=(0, w01 * 64),
                                skip_group_check=True,
                            )
                    if has_vbias:
                        for w01 in (0, 1):
                            nc.tensor.matmul(
                                v_ps[w01 * 64 : w01 * 64 + N, :],
                                onep_sb[:, 0:N],
                                bv_sb[:, :],
                                start=False,
                                stop=True,
                                tile_position=(0, w01 * 64),
                                skip_group_check=True,
                            )
                    v_sb = v_pool.tile([128, HEADS, HD + 1], f16, tag="vsb")
                    if CFG["vcopy"] == "scalar":
                        nc.scalar.activation(
                            out=v_sb[:, :, 0:HD],
                            in_=v_ps[:, :].rearrange("p (h d) -> p h d", d=HD),
                            func=AFT.Copy,
                        )
                    else:
                        nc.vector.tensor_copy(
                            v_sb[:, :, 0:HD],
                            v_ps[:, :].rearrange("p (h d) -> p h d", d=HD),
                        )
                    nc.vector.memset(v_sb[:, :, HD : HD + 1], 1.0)
                    v_tiles.append(v_sb)

                # --- scores + softmax numerator ---
                # head-pair (hp, hp+8): same row strip 32*(hp%4), one bank
                e_tiles = [None] * (HEADS // 2)
                for hp in range(HEADS // 2):
                    strip = 32 * (hp % 4)
                    s_ps = ps1.tile([128, 2, PAIRS, N], f32, tag="ps1")
                    if oct_i == 0 and hp < CFG["ps1"]:
                        # first use of each slot: zero the partition gaps
                        # (49-63, 113-127) that exp reads but no MM writes
                        nc.vector.memset(s_ps, 0.0)
                    for pr in range(PAIRS):
                        for w01 in (0, 1):
                            c0 = (pr * 2 + w01) * N
                            for j in (0, 1):
                                quad = hp // 4 + 2 * j
                                nc.tensor.matmul(
                                    s_ps[w01 * 64 : w01 * 64 + N, j, pr, :],
                                    kT[strip : strip + 32, quad, c0 : c0 + N],
                                    qT[strip : strip + 32, quad, c0 : c0 + N],
                                    start=True,
                                    stop=True,
                                    tile_position=(strip, w01 * 64),
                                )
                    e_sb = e_pool.tile([128, 2, PAIRS, N], f16, tag="esb")
                    nc.scalar.activation(
                        out=e_sb[:, :, :, :], in_=s_ps[:, :, :, :], func=AFT.Exp
                    )
                    _ebeng = getattr(nc, CFG["ebmul"])
                    _ebeng.tensor_mul(
                        e_sb[:, :, :, :],
                        e_sb[:, :, :, :],
                        eb_sb[:, hp, :, None, :].to_broadcast([128, 2, PAIRS, N]),
                    )
                    e_tiles[hp] = e_sb

                # --- context + normalize + store ---
                for pr in range(PAIRS):
                    out_sb = o_pool.tile([128, 2, HEADS, HD], f16, tag="osb")
                    v_sb = v_tiles[pr]
                    for half in (0, 1):
                        c_ps = ps2.tile([128, 2, DIM], f32, tag="ps2")
                        for hh in range(8):
                            h = half * 8 + hh
                            hp, j = h % 8, h // 8
                            for w01 in (0, 1):
                                b_ = w01 * 64
                                nc.tensor.matmul(
                                    c_ps[b_ : b_ + N, w01, 33 * hh : 33 * hh + 33],
                                    e_tiles[hp][b_ : b_ + N, j, pr, :],
                                    v_sb[b_ : b_ + N, h, :],
                                    start=True,
                                    stop=True,
                                    tile_position=(b_, b_),
                                )
                        cv = c_ps[:, :, 0 : 8 * 33].rearrange(
                            "p w (h e) -> p w h e", e=33
                        )
                        rec = small.tile([128, 2, 8, 1], f32, tag="rec")
                        nc.vector.reciprocal(rec[:, :, :, :], cv[:, :, :, HD : HD + 1])
                        nc.vector.tensor_mul(
                            out_sb[:, :, half * 8 : half * 8 + 8, :],
                            cv[:, :, :, 0:HD],
                            rec[:, :, :, :].to_broadcast([128, 2, 8, HD]),
                        )
                    r0 = t0 + pr * 2 * N
                    nc.sync.dma_start(
                        out=out_d[r0 : r0 + N, :],
                        in_=out_sb[0:N, 0, :, :].rearrange("p h d -> p (h d)"),
                    )
                    nc.sync.dma_start(
                        out=out_d[r0 + N : r0 + 2 * N, :],
                        in_=out_sb[64 : 64 + N, 1, :, :].rearrange("p h d -> p (h d)"),
                    )

    nc.compile()
    return nc


def _host_prep(hidden_states, Wq, bq, Wk, bk, Wv, bv, rel_pos_bias_table, rel_pos_index):
    scale = float(HD) ** -0.5
    x = np.asarray(hidden_states, dtype=np.float32).reshape(B * N, DIM)
    wq = np.ascontiguousarray((np.asarray(Wq, dtype=np.float32) * scale).astype(np.float16))
    wk = np.ascontiguousarray(np.asarray(Wk, dtype=np.float32).astype(np.float16))
    wv = np.ascontiguousarray(np.asarray(Wv, dtype=np.float16))
    bqs = (np.asarray(bq, dtype=np.float32) * scale).reshape(4, 128).T.copy()
    bks = np.asarray(bk, dtype=np.float32).reshape(4, 128).T.copy()
    bvs = np.asarray(bv, dtype=np.float16).reshape(1, DIM).copy()

    table = np.asarray(rel_pos_bias_table, dtype=np.float32)
    idx = np.asarray(rel_pos_index, dtype=np.int64)
    bias = table[idx.reshape(-1)].reshape(N, N, HEADS)  # [q, k, h]
    biasT = np.exp(bias.transpose(2, 1, 0))  # exp, [h, k, q]
    # head-paired layout: [p, hp, j, q] with h = hp + 8j
    eb = np.zeros((128, HEADS // 2, 2, N), dtype=np.float16)
    for hp in range(HEADS // 2):
        for j in range(2):
            eb[0:N, hp, j, :] = biasT[hp + 8 * j]
            eb[64 : 64 + N, hp, j, :] = biasT[hp + 8 * j]

    has_vbias = bool(
        np.abs(np.asarray(bv, dtype=np.float32)).max() > 0
        or np.abs(np.asarray(bq, dtype=np.float32)).max() > 0
        or np.abs(np.asarray(bk, dtype=np.float32)).max() > 0
    )
    xT = np.ascontiguousarray(x.T.astype(np.float16))  # [DIM, B*N] f16
    in_maps = []
    for c in range(NCORES):
        in_maps.append(
            {
                "xt": np.ascontiguousarray(xT[:, c * T : (c + 1) * T]),
                "wq": wq,
                "wk": wk,
                "wv": wv,
                "bq": bqs,
                "bk": bks,
                "bv": bvs,
                "expb": eb,
            }
        )
    return in_maps, has_vbias


def kernel(hidden_states, Wq, bq, Wk, bk, Wv, bv, rel_pos_bias_table, rel_pos_index):
    from concourse.bass_utils import run_bass_kernel_spmd

    in_maps, has_vbias = _host_prep(
        hidden_states, Wq, bq, Wk, bk, Wv, bv, rel_pos_bias_table, rel_pos_index
    )
    if has_vbias not in _NC_CACHE:
        _NC_CACHE[has_vbias] = _build_nc(has_vbias)
    nc = _NC_CACHE[has_vbias]

    res = run_bass_kernel_spmd(nc, in_maps, core_ids=list(range(NCORES)))
    out = np.empty((B * N, DIM), dtype=np.float32)
    for c in range(NCORES):
        out[c * T : (c + 1) * T] = res.results[c]["out"]
    return out.reshape(B, N, DIM)


# revision 14
# speedup vs baseline: 1.0948x; 1.0948x over previous
"""DonutSwin window self-attention on 8 Trainium2 NeuronCores.

Strategy (data-parallel over windows, 512 windows/core):
- Host: shard hidden_states over cores, pre-transpose + cast each shard to
  xT f16 [512, 25088] (feature-major), fold the 1/sqrt(hd) scale into Wq,
  cast weights to f16, precompute exp(rel-pos-bias)^T tiles (head-paired
  layout: pair (hp, hp+8) shares a PE row strip).
- Device per core, per 8-window block:
  * qT/kT = W^T @ xT via f16 matmuls (feature-major, head-dim on
    partitions); FWL hides the 128-col weight loads. oc order 0,2,1,3 so
    early head-quads unblock scores sooner.
  * v = x @ Wv via f16 matmuls over 49-token windows into a
    {win0: partitions 0-48, win1: 64-112} pair layout (2x col-tile
    concurrency via tile_position).
  * scores^T[k,q]: per head-pair (hp, hp+8) ONE PSUM bank [128, 2, 4, 49]
    holds all 8 windows x 2 heads; both heads use row strip 32*(hp%4), so
    the one-row-strip-per-bank PE rule holds; win0/win1 go to col halves.
  * softmax: one exp per head-pair on ACT (FD=392); multiply by
    exp(bias)^T on GPSIMD (frees DVE); row-sums via an appended
    ones-column in V.
  * ctx = P@V per (window-pair, head-half) into 2-bank PSUM tiles
    (win0 -> bank A, win1 -> bank B), normalized on DVE by recip sums.
- PSUM pools double as scheduling fences (mirrors the tuned baseline):
  one 4-buf pool shared by qk+scores, one 2-buf 2-bank pool by v+ctx.
- Output gathered to [4096, 49, 512] fp32.
"""

import numpy as np

WIN = 7
DIM = 512
HEADS = 16
HD = DIM // HEADS  # 32
B = 4096
N = WIN * WIN  # 49
NCORES = 8
BC = B // NCORES  # 512 windows per core
T = BC * N  # 25088 tokens per core
OCT = 8  # windows per block
NOCT = BC // OCT  # 64
TOK_OCT = OCT * N  # 392
PAIRS = OCT // 2  # 4 window-pairs per block

_NC_CACHE = {}
CFG = {
    "ebmul": "vector",  # engine for exp(bias) multiply: gpsimd | vector
    "qcopy": "scalar",
    "kcopy": "vector",
    "vcopy": "scalar",
    "xt_bufs": 4,
    "qk_bufs": 3,
    "e_bufs": 12,
    "o_bufs": 6,
    "v_bufs": 8,
    "ps1": 4,
    "ps2": 2,
}


def _build_nc(has_vbias, noct=NOCT):
    import concourse.bass as bass  # noqa: F401
    import concourse.tile as tile
    from concourse import bacc, mybir

    f32 = mybir.dt.float32
    f16 = mybir.dt.float16
    AFT = mybir.ActivationFunctionType

    nc = bacc.Bacc("TRN2", target_bir_lowering=False, debug=False)

    T_ = noct * TOK_OCT
    xT_d = nc.dram_tensor("xt", [DIM, T_], f16, kind="ExternalInput")
    wq_d = nc.dram_tensor("wq", [DIM, DIM], f16, kind="ExternalInput")
    wk_d = nc.dram_tensor("wk", [DIM, DIM], f16, kind="ExternalInput")
    wv_d = nc.dram_tensor("wv", [DIM, DIM], f16, kind="ExternalInput")
    bq_d = nc.dram_tensor("bq", [128, 4], f32, kind="ExternalInput")
    bk_d = nc.dram_tensor("bk", [128, 4], f32, kind="ExternalInput")
    bv_d = nc.dram_tensor("bv", [1, DIM], f16, kind="ExternalInput")
    # exp(bias)^T head-paired: [p, hp, j, q] = exp(bias[h=hp+8j].T)[k=p, q]
    # rows 0-48 and 64-112 both hold the same table (win0/win1 halves).
    eb_d = nc.dram_tensor("expb", [128, HEADS // 2, 2, N], f16, kind="ExternalInput")
    out_d = nc.dram_tensor("out", [T_, DIM], f16, kind="ExternalOutput")

    with tile.TileContext(nc) as tc:
        with (
            tc.tile_pool(name="consts", bufs=1) as consts,
            tc.tile_pool(name="xt", bufs=CFG["xt_bufs"]) as xt_pool,
            tc.tile_pool(name="qk", bufs=CFG["qk_bufs"]) as qk_pool,
            tc.tile_pool(name="vsb", bufs=CFG["v_bufs"]) as v_pool,
            tc.tile_pool(name="esb", bufs=CFG["e_bufs"]) as e_pool,
            tc.tile_pool(name="osb", bufs=CFG["o_bufs"]) as o_pool,
            tc.tile_pool(name="small", bufs=10) as small,
            tc.tile_pool(name="ps1", bufs=CFG["ps1"], space="PSUM") as ps1,
            tc.tile_pool(name="ps2", bufs=CFG["ps2"], space="PSUM") as ps2,
        ):
            wq_sb = consts.tile([128, 4, DIM], f16, tag="wq")
            wk_sb = consts.tile([128, 4, DIM], f16, tag="wk")
            wv_sb = consts.tile([128, 4, DIM], f16, tag="wv")
            for w_sb, w_d in ((wq_sb, wq_d), (wk_sb, wk_d), (wv_sb, wv_d)):
                nc.sync.dma_start(
                    out=w_sb, in_=w_d[:, :].rearrange("(i p) o -> p i o", p=128)
                )
            bq_sb = consts.tile([128, 4], f32, tag="bq")
            bk_sb = consts.tile([128, 4], f32, tag="bk")
            nc.sync.dma_start(out=bq_sb, in_=bq_d[:, :])
            nc.sync.dma_start(out=bk_sb, in_=bk_d[:, :])
            eb_sb = consts.tile([128, HEADS // 2, 2, N], f16, tag="eb")
            nc.sync.dma_start(out=eb_sb, in_=eb_d[:, :, :, :])
            if has_vbias:
                onep_sb = consts.tile([1, 2 * N], f16, tag="onep")
                nc.vector.memset(onep_sb, 1.0)
                bv_sb = consts.tile([1, DIM], f16, tag="bv")
                nc.sync.dma_start(out=bv_sb, in_=bv_d[:, :])

            OC_ORDER = (0, 2, 1, 3)  # early quads (0, 2) unblock hp 0-3 first

            for oct_i in range(noct):
                t0 = oct_i * TOK_OCT
                xt = xt_pool.tile([128, 4, TOK_OCT], f16, tag="xt")
                nc.sync.dma_start(
                    out=xt,
                    in_=xT_d[:, t0 : t0 + TOK_OCT].rearrange("(i p) t -> p i t", p=128),
                )

                # --- q^T / k^T projections (feature-major, f16 + FWL) ---
                qT = qk_pool.tile([128, 4, TOK_OCT], f16, tag="qT")
                kT = qk_pool.tile([128, 4, TOK_OCT], f16, tag="kT")
                for dst, w_sb, b_sb, on_act in (
                    (qT, wq_sb, bq_sb, True),
                    (kT, wk_sb, bk_sb, False),
                ):
                    for oc in OC_ORDER:
                        ps = ps1.tile([128, TOK_OCT], f32, tag="ps1")
                        for ic in range(4):
                            nc.tensor.matmul(
                                ps[:, :],
                                w_sb[:, ic, oc * 128 : (oc + 1) * 128],
                                xt[:, ic, :],
                                start=(ic == 0),
                                stop=(ic == 3),
                            )
                        if on_act:
                            if has_vbias:
                                nc.scalar.activation(
                                    out=dst[:, oc, :],
                                    in_=ps[:, :],
                                    func=AFT.Identity,
                                    bias=b_sb[:, oc : oc + 1],
                                )
                            else:
                                nc.scalar.activation(
                                    out=dst[:, oc, :], in_=ps[:, :], func=AFT.Copy
                                )
                        elif has_vbias:
                            nc.vector.tensor_scalar_add(
                                dst[:, oc, :], ps[:, :], b_sb[:, oc : oc + 1]
                            )
                        else:
                            nc.vector.tensor_copy(dst[:, oc, :], ps[:, :])

                # --- v projection: col-tiled into {0-48, 64-112} pair layout ---
                v_tiles = []
                for pr in range(PAIRS):
                    v2 = ps2.tile([128, 2, DIM], f32, tag="ps2")
                    v_ps = v2[:, 0, :]
                    for ic in range(4):
                        for w01 in (0, 1):
                            c0 = (pr * 2 + w01) * N
                            nc.tensor.matmul(
                                v_ps[w01 * 64 : w01 * 64 + N, :],
                                xt[:, ic, c0 : c0 + N],
                                wv_sb[:, ic, :],
                                start=(ic == 0),
                                stop=(ic == 3) and not has_vbias,
                                tile_position=(0, w01 * 64),
                                skip_group_check=True,
                            )
                    if has_vbias:
                        for w01 in (0, 1):
                            nc.tensor.matmul(
                                v_ps[w01 * 64 : w01 * 64 + N, :],
                                onep_sb[:, 0:N],
                                bv_sb[:, :],
                                start=False,
                                stop=True,
                                tile_position=(0, w01 * 64),
                                skip_group_check=True,
                            )
                    v_sb = v_pool.tile([128, HEADS, HD + 1], f16, tag="vsb")
                    if CFG["vcopy"] == "scalar":
                        nc.scalar.activation(
                            out=v_sb[:, :, 0:HD],
                            in_=v_ps[:, :].rearrange("p (h d) -> p h d", d=HD),
                            func=AFT.Copy,
                        )
                    else:
                        nc.vector.tensor_copy(
                            v_sb[:, :, 0:HD],
                            v_ps[:, :].rearrange("p (h d) -> p h d", d=HD),
                        )
                    nc.vector.memset(v_sb[:, :, HD : HD + 1], 1.0)
                    v_tiles.append(v_sb)

                # --- scores + softmax numerator ---
                # head-pair (hp, hp+8): same row strip 32*(hp%4), one bank
                e_tiles = [None] * (HEADS // 2)
                for hp in range(HEADS // 2):
                    strip = 32 * (hp % 4)
                    s_ps = ps1.tile([128, 2, PAIRS, N], f32, tag="ps1")
                    if oct_i == 0 and hp < CFG["ps1"]:
                        # first use of each slot: zero the partition gaps
                        # (49-63, 113-127) that exp reads but no MM writes
                        nc.vector.memset(s_ps, 0.0)
                    for pr in range(PAIRS):
                        for w01 in (0, 1):
                            c0 = (pr * 2 + w01) * N
                            for j in (0, 1):
                                quad = hp // 4 + 2 * j
                                nc.tensor.matmul(
                                    s_ps[w01 * 64 : w01 * 64 + N, j, pr, :],
                                    kT[strip : strip + 32, quad, c0 : c0 + N],
                                    qT[strip : strip + 32, quad, c0 : c0 + N],
                                    start=True,
                                    stop=True,
                                    tile_position=(strip, w01 * 64),
                                )
                    e_sb = e_pool.tile([128, 2, PAIRS, N], f16, tag="esb")
                    nc.scalar.activation(
                        out=e_sb[:, :, :, :], in_=s_ps[:, :, :, :], func=AFT.Exp
                    )
                    _ebeng = getattr(nc, CFG["ebmul"])
                    _ebeng.tensor_mul(
                        e_sb[:, :, :, :],
                        e_sb[:, :, :, :],
                        eb_sb[:, hp, :, None, :].to_broadcast([128, 2, PAIRS, N]),
                    )
                    e_tiles[hp] = e_sb

                # --- context + normalize + store ---
                for pr in range(PAIRS):
                    out_sb = o_pool.tile([128, 2, HEADS, HD], f16, tag="osb")
                    v_sb = v_tiles[pr]
                    for half in (0, 1):
                        c_ps = ps2.tile([128, 2, DIM], f32, tag="ps2")
                        for hh in range(8):
                            h = half * 8 + hh
                            hp, j = h % 8, h // 8
                            for w01 in (0, 1):
                                b_ = w01 * 64
                                nc.tensor.matmul(
                                    c_ps[b_ : b_ + N, w01, 33 * hh : 33 * hh + 33],
                                    e_tiles[hp][b_ : b_ + N, j, pr, :],
                                    v_sb[b_ : b_ + N, h, :],
                                    start=True,
                                    stop=True,
                                    tile_position=(b_, b_),
                                )
                        cv = c_ps[:, :, 0 : 8 * 33].rearrange(
                            "p w (h e) -> p w h e", e=33
                        )
                        rec = small.tile([128, 2, 8, 1], f32, tag="rec")
                        nc.vector.reciprocal(rec[:, :, :, :], cv[:, :, :, HD : HD + 1])
                        nc.vector.tensor_mul(
                            out_sb[:, :, half * 8 : half * 8 + 8, :],
                            cv[:, :, :, 0:HD],
                            rec[:, :, :, :].to_broadcast([128, 2, 8, HD]),
                        )
                    r0 = t0 + pr * 2 * N
                    nc.sync.dma_start(
                        out=out_d[r0 : r0 + N, :],
                        in_=out_sb[0:N, 0, :, :].rearrange("p h d -> p (h d)"),
                    )
                    nc.sync.dma_start(
                        out=out_d[r0 + N : r0 + 2 * N, :],
                        in_=out_sb[64 : 64 + N, 1, :, :].rearrange("p h d -> p (h d)"),
                    )

    nc.compile()
    return nc


def _host_prep(hidden_states, Wq, bq, Wk, bk, Wv, bv, rel_pos_bias_table, rel_pos_index):
    scale = float(HD) ** -0.5
    x = np.asarray(hidden_states, dtype=np.float32).reshape(B * N, DIM)
    wq = np.ascontiguousarray((np.asarray(Wq, dtype=np.float32) * scale).astype(np.float16))
    wk = np.ascontiguousarray(np.asarray(Wk, dtype=np.float32).astype(np.float16))
    wv = np.ascontiguousarray(np.asarray(Wv, dtype=np.float16))
    bqs = (np.asarray(bq, dtype=np.float32) * scale).reshape(4, 128).T.copy()
    bks = np.asarray(bk, dtype=np.float32).reshape(4, 128).T.copy()
    bvs = np.asarray(bv, dtype=np.float16).reshape(1, DIM).copy()

    table = np.asarray(rel_pos_bias_table, dtype=np.float32)
    idx = np.asarray(rel_pos_index, dtype=np.int64)
    bias = table[idx.reshape(-1)].reshape(N, N, HEADS)  # [q, k, h]
    biasT = np.exp(bias.transpose(2, 1, 0))  # exp, [h, k, q]
    # head-paired layout: [p, hp, j, q] with h = hp + 8j
    eb = np.zeros((128, HEADS // 2, 2, N), dtype=np.float16)
    for hp in range(HEADS // 2):
        for j in range(2):
            eb[0:N, hp, j, :] = biasT[hp + 8 * j]
            eb[64 : 64 + N, hp, j, :] = biasT[hp + 8 * j]

    has_vbias = bool(
        np.abs(np.asarray(bv, dtype=np.float32)).max() > 0
        or np.abs(np.asarray(bq, dtype=np.float32)).max() > 0
        or np.abs(np.asarray(bk, dtype=np.float32)).max() > 0
    )
    xT = np.ascontiguousarray(x.T.astype(np.float16))  # [DIM, B*N] f16
    in_maps = []
    for c in range(NCORES):
        in_maps.append(
            {
                "xt": np.ascontiguousarray(xT[:, c * T : (c + 1) * T]),
                "wq": wq,
                "wk": wk,
                "wv": wv,
                "bq": bqs,
                "bk": bks,
                "bv": bvs,
                "expb": eb,
            }
        )
    return in_maps, has_vbias


def kernel(hidden_states, Wq, bq, Wk, bk, Wv, bv, rel_pos_bias_table, rel_pos_index):
    from concourse.bass_utils import run_bass_kernel_spmd

    in_maps, has_vbias = _host_prep(
        hidden_states, Wq, bq, Wk, bk, Wv, bv, rel_pos_bias_table, rel_pos_index
    )
    if has_vbias not in _NC_CACHE:
        _NC_CACHE[has_vbias] = _build_nc(has_vbias)
    nc = _NC_CACHE[has_vbias]

    res = run_bass_kernel_spmd(nc, in_maps, core_ids=list(range(NCORES)))
    out = np.empty((B * N, DIM), dtype=np.float32)
    for c in range(NCORES):
        out[c * T : (c + 1) * T] = res.results[c]["out"]
    return out.reshape(B, N, DIM)
